# revision 16
# baseline (speedup 1.0000x reference)
"""Trainium2 Bass kernel: sparse (sliding-window) attention block, v2.

Full module per reference:
  RMSNorm -> fused QKV (5120x2880) -> YaRN RoPE -> GQA sliding-window(128)
  causal attention with learned sink logit -> out projection (2880x4096).

Sharding: tensor-parallel over heads across 8 cores. Core c owns q-heads
{8c + dev} for dev in DEV_ORDER and kv-head c. Each core emits a partial
[1024, 2880] bf16 output; the host sums the partials and adds out_b.

v2 structure (vs v1 baseline):
  - sum-of-squares on the vector engine (balanced bf16 tree) + gpsimd
    partition_all_reduce; rsqrt via scalar Sqrt + vector reciprocal.
    PE no longer runs the 46 ssq matmuls.
  - QKV bias+rsq handled without blocking the psum drain: qkvT holds the
    RAW (unscaled, no-bias) projection in bf16; rsq is folded into the
    rope tables (cos*rsq, sin*rsq) and the bias contribution is a
    host-precomputed additive rope table (b_rope).  v path applies
    (raw*rsq + b) explicitly before the token-major transpose.
  - device head order [0,2,4,6,1,3,5,7]: AV matmuls use V as the
    stationary operand and write [head-pair x 64 dims, tok] directly in
    the out-proj lhsT layout (no per-head transposes).
  - sm_scale folded into the Exp activation scale; masks multiplicative
    {0,1} bf16 applied post-exp.
  - softmax denominators via gpsimd partition_all_reduce over the kt
    partitions (+ sink term), reciprocal in place, single normalize mul.
  - y written bf16 (host accumulates in f64).
"""

import math
import sys

import numpy as np

try:
    import concourse.bass as bass
except ImportError:  # pragma: no cover
    sys.path.insert(0, "/opt/trn_rl_repo")
    import concourse.bass as bass

import concourse.bacc as bacc
import concourse.tile as tile
from concourse import bass_isa, mybir
from concourse.masks import make_identity
from concourse.bass_utils import run_bass_kernel_spmd

import ml_dtypes

BF16 = ml_dtypes.bfloat16

T = 1024
HIDDEN = 2880
HD = 64
NH = 64
NKV = 8
SW = 128
NCORES = 8
HPC = NH // NCORES          # q heads per core = 8
QKV_DIM = HD * (NH + 2 * NKV)
SM_SCALE = 1.0 / math.sqrt(HD)

P = 128
KT = (HIDDEN + P - 1) // P   # 23 k-tiles over hidden (zero-padded to 2944)
KPAD = KT * P
NT = 5                       # qkv n-tiles of 128 (4 q-tiles + 1 kv-tile)
MT = T // P                  # 8 token tiles

# device head order: even heads first, odd second, so that AV head-pair
# stacking (partitions 0:64 / 64:128 of free-slot kk) reproduces the
# baseline out_w tile layout (orig heads 2kk, 2kk+1).
DEV_ORDER = [0, 2, 4, 6, 1, 3, 5, 7]

dt = mybir.dt

_CACHE = {}


# ----------------------------------------------------------------------------
# host-side helpers
# ----------------------------------------------------------------------------

def _rope_cos_sin(num_tokens):
    base = 150000.0
    scaling = 32.0
    init_ctx = 4096.0
    ntk_alpha = 1.0
    ntk_beta = 32.0
    d_half = HD / 2
    freq = base ** (np.arange(0, HD, 2, dtype=np.float32) / HD)
    concentration = 0.1 * math.log(scaling) + 1.0
    low = d_half * math.log(init_ctx / (ntk_beta * 2 * math.pi)) / math.log(base)
    high = d_half * math.log(init_ctx / (ntk_alpha * 2 * math.pi)) / math.log(base)
    interpolation = 1.0 / (scaling * freq)
    extrapolation = 1.0 / freq
    ramp = (np.arange(int(d_half), dtype=np.float32) - low) / (high - low)
    m = 1.0 - np.clip(ramp, 0.0, 1.0)
    inv_freq = interpolation * (1.0 - m) + extrapolation * m
    t = np.arange(num_tokens, dtype=np.float32)
    freqs = t[:, None] * inv_freq[None, :]
    cos = (np.cos(freqs) * concentration).astype(np.float32)
    sin = (np.sin(freqs) * concentration).astype(np.float32)
    return cos, sin  # [T, 32]


def _host_tables():
    """Plain (unscaled) replicated rope tables with the swap sign folded
    into sin: rope(u)[p] = u[p]*cos[p] + u[p^32]*sin_alt[p]."""
    cos, sin = _rope_cos_sin(T)  # [1024, 32]
    sgn = np.repeat([-1.0, 1.0], 32)[:, None].astype(np.float32)
    sgn = np.tile(sgn, (2, 1))  # [128, 1]
    cos_t = np.tile(cos.T, (4, 1)).astype(np.float32)          # [128, 1024]
    sin_t = (np.tile(sin.T, (4, 1)) * sgn).astype(np.float32)  # [128, 1024]
    return cos_t, sin_t


def _host_masks01():
    j = np.arange(P)[:, None]   # kt row (partition)
    i = np.arange(P)[None, :]   # q col (free)
    mask_prev = (j > i).astype(np.float32)    # dist in [1,127]
    mask_self = (j <= i).astype(np.float32)   # dist in [0,127]
    return np.concatenate([mask_prev, mask_self], axis=1).astype(BF16)


def _prep_core_inputs(core, x, norm_scale, qkv_w, qkv_b, out_w, sinks):
    q_end = NH * HD
    k_end = q_end + NKV * HD

    heads = [core * HPC + d for d in DEV_ORDER]
    qrows = np.concatenate([np.arange(h * HD, (h + 1) * HD) for h in heads])
    krows = np.arange(q_end + core * HD, q_end + (core + 1) * HD)
    vrows = np.arange(k_end + core * HD, k_end + (core + 1) * HD)
    rows = np.concatenate([qrows, vrows, krows])  # [640]

    wshard = (qkv_w[rows, :] * norm_scale[None, :]).astype(np.float32)
    bshard = qkv_b[rows].astype(np.float32)  # [640]

    # lhsT tiles: wq[n, kp, kt*128 + nc] = wshard[n*128 + nc, kt*128 + kp]
    wq = np.zeros((NT, P, KPAD), dtype=BF16)
    for n in range(NT):
        blk = wshard[n * P:(n + 1) * P, :]  # [128 n, 2880 k]
        for ki in range(KT):
            k0 = ki * P
            ksz = min(P, HIDDEN - k0)
            wq[n, :ksz, ki * P:ki * P + P] = blk[:, k0:k0 + ksz].T.astype(BF16)

    # out_w shard (orig head order preserved; see module docstring)
    cols = np.arange(core * HPC * HD, (core + 1) * HPC * HD)
    wo = out_w[:, cols].T.astype(np.float32)  # [512 hd, 2880 H]
    wout = wo.reshape(4, P, HIDDEN).astype(BF16)

    bqkv = bshard.reshape(NT, P).T.copy().astype(np.float32)  # [128, 5]

    cos_t, sin_t = _host_tables()  # [128, 1024] f32 each

    # additive rope bias tables: brope[p, n, t] = b_n[p]*cos[p,t] + b_n[p^32]*sin_alt[p,t]
    brope = np.zeros((P, NT, T), dtype=np.float32)
    for n in range(NT):
        b = bqkv[:, n]                       # [128]
        b_swp = b[np.arange(P) ^ 32]
        brope[:, n, :] = b[:, None] * cos_t + b_swp[:, None] * sin_t
    brope = brope.astype(BF16)

    # padded transposed x
    xt = np.zeros((KPAD, T), dtype=BF16)
    xt[:HIDDEN] = x.T.astype(BF16)

    esink = np.exp(sinks[heads].astype(np.float64)).astype(np.float32)
    esink = np.broadcast_to(esink, (P, HPC)).copy()

    return {
        "xt": xt,                               # [2944, 1024] bf16
        "wq": wq,                               # [5, 128, 2944] bf16
        "wout": wout,                           # [4, 128, 2880] bf16
        "bqkv": bqkv,                           # [128, 5] f32
        "cos_t": cos_t, "sin_t": sin_t,         # [128, 1024] f32
        "brope": brope,                         # [128, 5, 1024] bf16
        "mask": _host_masks01(),                # [128, 256] bf16
        "esink": esink,                         # [128, 8] f32
    }


# ----------------------------------------------------------------------------
# device kernel (Tile)
# ----------------------------------------------------------------------------

def build_nc():
    nc = bacc.Bacc("TRN2", target_bir_lowering=False, debug=False)

    xt_d = nc.dram_tensor("xt", [KPAD, T], dt.bfloat16, kind="ExternalInput").ap()
    wq_d = nc.dram_tensor("wq", [NT, P, KPAD], dt.bfloat16, kind="ExternalInput").ap()
    wout_d = nc.dram_tensor("wout", [4, P, HIDDEN], dt.bfloat16, kind="ExternalInput").ap()
    bqkv_d = nc.dram_tensor("bqkv", [P, NT], dt.float32, kind="ExternalInput").ap()
    cos_d = nc.dram_tensor("cos_t", [P, T], dt.float32, kind="ExternalInput").ap()
    sin_d = nc.dram_tensor("sin_t", [P, T], dt.float32, kind="ExternalInput").ap()
    brope_d = nc.dram_tensor("brope", [P, NT, T], dt.bfloat16, kind="ExternalInput").ap()
    mask_d = nc.dram_tensor("mask", [P, 2 * P], dt.bfloat16, kind="ExternalInput").ap()
    esink_d = nc.dram_tensor("esink", [P, HPC], dt.float32, kind="ExternalInput").ap()
    y_d = nc.dram_tensor("y", [T, HIDDEN], dt.bfloat16, kind="ExternalOutput").ap()

    YC = 480                     # out-proj psum chunk width (6 chunks of 480)

    def bcast_mid(ap2d, n):
        """[P, F] -> [P, n, F] with a 0-step middle dim (free broadcast)."""
        return bass.AP(tensor=ap2d.tensor, offset=ap2d.offset,
                       ap=[ap2d.ap[0], [0, n]] + list(ap2d.ap[1:]))

    def bcast_last(ap2d, n):
        """[P, F] -> [P, F, n] with a 0-step inner dim (free broadcast)."""
        return bass.AP(tensor=ap2d.tensor, offset=ap2d.offset,
                       ap=list(ap2d.ap) + [[0, n]])

    with tile.TileContext(nc) as tc:
        with (
            tc.tile_pool(name="const", bufs=1) as const,
            tc.tile_pool(name="res", bufs=1) as res,
            tc.tile_pool(name="qkvp", bufs=2) as qkvp,
            tc.tile_pool(name="xsqp", bufs=2) as xsqp,
            tc.tile_pool(name="ropep", bufs=2) as ropep,
            tc.tile_pool(name="ptp", bufs=2) as ptp,
            tc.tile_pool(name="denp", bufs=2) as denp,
            tc.tile_pool(name="attp", bufs=2) as attp,
            tc.tile_pool(name="ysbp", bufs=3) as ysbp,
            tc.tile_pool(name="pA", bufs=4, space="PSUM") as pA,
            tc.tile_pool(name="pS", bufs=2, space="PSUM") as pS,
            tc.tile_pool(name="pAV", bufs=2, space="PSUM") as pAV,
        ):
            # ---- constants ----
            zbias = const.tile([P, 1], dt.float32, tag="zbias", name="zbias")
            nc.vector.memset(zbias, 0.0)
            eps_t = const.tile([P, 1], dt.float32, tag="eps", name="eps_t")
            nc.vector.memset(eps_t, 1e-5)
            identb = const.tile([P, P], dt.bfloat16, tag="identb", name="identb")
            make_identity(nc, identb)
            # prefetch both activation tables (Sqrt, Exp) with dummy ops
            dmy = const.tile([1, 2], dt.float32, tag="dmy", name="dmy")
            nc.scalar.activation(dmy[:, 0:1], zbias[0:1, :],
                                 mybir.ActivationFunctionType.Sqrt,
                                 bias=zbias[0:1, :])
            nc.scalar.activation(dmy[:, 1:2], zbias[0:1, :],
                                 mybir.ActivationFunctionType.Exp,
                                 bias=zbias[0:1, :])

            # ---- DMA issue (sync queue) ----
            wq_sb = []
            for n in range(NT):
                w = res.tile([P, KPAD], dt.bfloat16, tag=f"wq{n}", name=f"wq{n}")
                wq_sb.append(w)
            xt_sb = res.tile([P, KT, T], dt.bfloat16, tag="xt", name="xt")
            wout_sb = []
            for kk in range(4):
                w = res.tile([P, HIDDEN], dt.bfloat16, tag=f"wout{kk}", name=f"wout{kk}")
                wout_sb.append(w)
            cos_sb = const.tile([P, T], dt.float32, tag="cos", name="cos_sb")
            sin_sb = const.tile([P, T], dt.float32, tag="sin", name="sin_sb")
            brope_sb = const.tile([P, NT, T], dt.bfloat16, tag="brope", name="brope_sb")
            mask_sb = const.tile([P, 2 * P], dt.bfloat16, tag="mask", name="mask_sb")
            esink_sb = const.tile([P, HPC], dt.float32, tag="esink", name="esink_sb")
            bqkv_sb = const.tile([P, NT], dt.float32, tag="bqkv", name="bqkv_sb")

            HK = KPAD // 2
            # kv weights first; interleave xt so the qkv4 chain can pace.
            nc.sync.dma_start(out=wq_sb[4][:, :HK], in_=wq_d[4, :, :HK])
            for ki in range(0, 4):
                nc.sync.dma_start(out=xt_sb[:, ki, :], in_=xt_d[ki * P:(ki + 1) * P, :])
            nc.sync.dma_start(out=wq_sb[4][:, HK:], in_=wq_d[4, :, HK:])
            for ki in range(4, 8):
                nc.sync.dma_start(out=xt_sb[:, ki, :], in_=xt_d[ki * P:(ki + 1) * P, :])
            nc.sync.dma_start(out=wq_sb[0][:, :HK], in_=wq_d[0, :, :HK])
            nc.sync.dma_start(out=wq_sb[0][:, HK:], in_=wq_d[0, :, HK:])
            for ki in range(8, 12):
                nc.sync.dma_start(out=xt_sb[:, ki, :], in_=xt_d[ki * P:(ki + 1) * P, :])
            HT = T // 2
            nc.sync.dma_start(out=cos_sb[:, :HT], in_=cos_d[:, :HT])
            nc.sync.dma_start(out=sin_sb[:, :HT], in_=sin_d[:, :HT])
            nc.sync.dma_start(out=cos_sb[:, HT:], in_=cos_d[:, HT:])
            nc.sync.dma_start(out=sin_sb[:, HT:], in_=sin_d[:, HT:])
            for n in range(NT):
                nc.sync.dma_start(out=brope_sb[:, n, :], in_=brope_d[:, n, :])
            nc.sync.dma_start(out=wq_sb[1][:, :HK], in_=wq_d[1, :, :HK])
            nc.sync.dma_start(out=wq_sb[1][:, HK:], in_=wq_d[1, :, HK:])
            for ki in range(12, 16):
                nc.sync.dma_start(out=xt_sb[:, ki, :], in_=xt_d[ki * P:(ki + 1) * P, :])
            HO = HIDDEN // 2
            nc.sync.dma_start(out=wout_sb[0][:, :HO], in_=wout_d[0, :, :HO])
            nc.sync.dma_start(out=wout_sb[0][:, HO:], in_=wout_d[0, :, HO:])
            nc.sync.dma_start(out=wq_sb[2][:, :HK], in_=wq_d[2, :, :HK])
            nc.sync.dma_start(out=wq_sb[2][:, HK:], in_=wq_d[2, :, HK:])
            for ki in range(16, 20):
                nc.sync.dma_start(out=xt_sb[:, ki, :], in_=xt_d[ki * P:(ki + 1) * P, :])
            for kk in range(1, 4):
                nc.sync.dma_start(out=wout_sb[kk][:, :HO], in_=wout_d[kk, :, :HO])
                nc.sync.dma_start(out=wout_sb[kk][:, HO:], in_=wout_d[kk, :, HO:])
            nc.sync.dma_start(out=wq_sb[3][:, :HK], in_=wq_d[3, :, :HK])
            nc.sync.dma_start(out=wq_sb[3][:, HK:], in_=wq_d[3, :, HK:])
            for ki in range(20, KT):
                nc.sync.dma_start(out=xt_sb[:, ki, :], in_=xt_d[ki * P:(ki + 1) * P, :])
            nc.sync.dma_start(out=mask_sb, in_=mask_d)
            nc.sync.dma_start(out=esink_sb, in_=esink_d)
            nc.sync.dma_start(out=bqkv_sb, in_=bqkv_d)

            # ---- sum of squares on vector (4 bf16 segment chains + f32 merge) ----
            acc = res.tile([P, T], dt.float32, tag="ssqacc", name="ssqacc")
            seg = [res.tile([P, T], dt.bfloat16, tag=f"seg{g}", name=f"seg{g}")
                   for g in range(4)]
            for ki in range(KT):
                g = min(ki // 6, 3)
                first = ki in (0, 6, 12, 18)
                if first:
                    nc.vector.tensor_mul(seg[g], xt_sb[:, ki, :], xt_sb[:, ki, :])
                else:
                    xsq = xsqp.tile([P, T], dt.bfloat16, tag="xsq", name="xsq")
                    nc.vector.tensor_mul(xsq, xt_sb[:, ki, :], xt_sb[:, ki, :])
                    nc.vector.tensor_add(seg[g], seg[g], xsq)
            nc.vector.tensor_add(acc, seg[0], seg[1])
            nc.vector.tensor_add(acc, acc, seg[2])
            nc.vector.tensor_add(acc, acc, seg[3])

            # rsq_b = 1/sqrt(mean + eps), broadcast on all 128 partitions
            rsq_b = res.tile([P, T], dt.float32, tag="rsq", name="rsq_b")
            nc.gpsimd.partition_all_reduce(rsq_b, acc, channels=P,
                                           reduce_op=bass_isa.ReduceOp.add)
            nc.scalar.activation(rsq_b, rsq_b,
                                 mybir.ActivationFunctionType.Sqrt,
                                 bias=eps_t, scale=1.0 / HIDDEN)
            nc.vector.reciprocal(rsq_b, rsq_b)
            # fold rsq into the rope tables (in place)
            nc.vector.tensor_mul(cos_sb, cos_sb, rsq_b)
            nc.vector.tensor_mul(sin_sb, sin_sb, rsq_b)

            # ---- qkv raw projection (bf16, no bias, no rsq) ----
            def qkv_tile(n, half, dst):
                """dst: [P, 512] sbuf bf16 destination for the raw projection."""
                c0 = half * 512
                pq = pA.tile([P, 512], dt.float32, tag="pb", name="pb")
                for ki in range(KT):
                    nc.tensor.matmul(
                        pq,
                        wq_sb[n][:, ki * P:ki * P + P],
                        xt_sb[:, ki, c0:c0 + 512],
                        start=(ki == 0), stop=(ki == KT - 1),
                    )
                nc.scalar.copy(dst, pq)

            # rope: dst = src*cosR + swp(src)*sinR + brope[n]; srct is [P, 512]
            def rope(srct, n, half, lo, hi, dsts):
                c0 = half * 512
                swp = ropep.tile([P, 512], dt.bfloat16, tag="swp", name="swp")
                for a in range(lo, hi, 32):
                    b = a ^ 32
                    nc.vector.tensor_copy(swp[a:a + 32, :], srct[b:b + 32, :])
                tc_ = ropep.tile([P, 512], dt.float32, tag="tc", name="tc")
                nc.vector.tensor_mul(tc_[lo:hi, :], srct[lo:hi, :],
                                     cos_sb[lo:hi, c0:c0 + 512])
                ts_ = ropep.tile([P, 512], dt.float32, tag="ts", name="ts")
                nc.vector.tensor_mul(ts_[lo:hi, :], swp[lo:hi, :],
                                     sin_sb[lo:hi, c0:c0 + 512])
                nc.vector.tensor_add(tc_[lo:hi, :], tc_[lo:hi, :], ts_[lo:hi, :])
                for i, dst in enumerate(dsts):
                    b0 = lo + 64 * i
                    nc.vector.tensor_add(dst, tc_[b0:b0 + 64, :],
                                         brope_sb[b0:b0 + 64, n, c0:c0 + 512])

            qra = res.tile([64, HPC, T], dt.bfloat16, tag="qra", name="qra")
            krope = res.tile([64, T], dt.bfloat16, tag="krope", name="krope")
            vtok = []
            for b in range(MT):
                vt = res.tile([P, HD], dt.bfloat16, tag=f"vtok{b}", name=f"vtok{b}")
                vtok.append(vt)

            def kv_half(half):
                c0 = half * 512
                qkvt = qkvp.tile([P, 512], dt.bfloat16, tag="qkvT", name="qkvt4")
                qkv_tile(4, half, qkvt)
                # v: (raw*rsq + b) -> bf16 token-major via PE transpose
                vsc = ropep.tile([P, 512], dt.float32, tag="tc", name="vsc")
                nc.vector.tensor_mul(vsc[0:64, :], qkvt[0:64, :],
                                     rsq_b[0:64, c0:c0 + 512])
                vs2 = ropep.tile([64, 512], dt.bfloat16, tag="vs2", name="vs2")
                nc.vector.tensor_scalar_add(vs2, vsc[0:64, :], bqkv_sb[0:64, 4:5])
                for j in range(4):
                    b = half * 4 + j
                    pv = pS.tile([P, HD], dt.bfloat16, tag="sc", name="pv")
                    nc.tensor.transpose(pv, vs2[:, j * P:(j + 1) * P],
                                        identb[:64, :64])
                    nc.vector.tensor_copy(vtok[b], pv)
                # k rope (rows 64:128)
                rope(qkvt, 4, half, 64, 128, [krope[:, c0:c0 + 512]])

            def q_half(half):
                c0 = half * 512
                for n in range(4):
                    qkvt = qkvp.tile([P, 512], dt.bfloat16, tag="qkvT", name=f"qkvt{n}")
                    qkv_tile(n, half, qkvt)
                    rope(qkvt, n, half, 0, 128,
                         [qra[:, 2 * n, c0:c0 + 512], qra[:, 2 * n + 1, c0:c0 + 512]])

            # ---- attention + out-projection for one token tile ----
            def attention(b):
                pt = ptp.tile([P, 2, HPC, P], dt.bfloat16, tag="pt", name="pt")
                kts = [(0, b - 1), (1, b)] if b > 0 else [(1, b)]
                for s, kt in kts:
                    for g in range(2):
                        ps = pS.tile([P, 4, P], dt.float32, tag="sc", name="sc")
                        nc.tensor.matmul(
                            ps,
                            krope[:, kt * P:(kt + 1) * P],
                            qra[:, 4 * g:4 * g + 4, b * P:(b + 1) * P],
                            start=True, stop=True)
                        nc.scalar.activation(pt[:, s, 4 * g:4 * g + 4, :], ps,
                                             mybir.ActivationFunctionType.Exp,
                                             bias=zbias, scale=SM_SCALE)
                    m0 = 0 if s == 0 else P
                    nc.vector.tensor_mul(pt[:, s], pt[:, s],
                                         bcast_mid(mask_sb[:, m0:m0 + P], HPC))
                # denominators: all-reduce over kt partitions, + sink, recip
                if b > 0:
                    ptsum = denp.tile([P, HPC, P], dt.float32, tag="ptsum",
                                      name="ptsum")
                    nc.vector.tensor_add(ptsum, pt[:, 0], pt[:, 1])
                    ar_in = ptsum
                else:
                    ar_in = pt[:, 1]
                den = denp.tile([P, HPC, P], dt.float32, tag="den", name="den")
                nc.gpsimd.partition_all_reduce(den, ar_in, channels=P,
                                               reduce_op=bass_isa.ReduceOp.add)
                for p0, h0 in ((0, 0), (64, 4)):
                    dsl = den[p0:p0 + 64, h0:h0 + 4, :]
                    nc.vector.tensor_add(dsl, dsl,
                                         bcast_last(esink_sb[p0:p0 + 64, h0:h0 + 4], P))
                    nc.vector.reciprocal(dsl, dsl)
                # AV: V stationary, probs moving; head-pairs stacked on partitions
                avp = pAV.tile([P, 4, P], dt.float32, tag="av", name="av")
                for g in range(2):
                    for idx, (s, kt) in enumerate(kts):
                        nc.tensor.matmul(
                            avp[64 * g:64 * g + 64, :, :],
                            vtok[kt],
                            pt[:, s, 4 * g:4 * g + 4, :],
                            start=(idx == 0), stop=(idx == len(kts) - 1))
                att = attp.tile([P, 4, P], dt.bfloat16, tag="att", name="att")
                nc.vector.tensor_mul(att[0:64], avp[0:64], den[0:64, 0:4, :])
                nc.vector.tensor_mul(att[64:128], avp[64:128], den[64:128, 4:8, :])

                # out projection: y[b] partial, bf16, 3 DMAs per tile
                for hy in range(3):
                    ysb = ysbp.tile([P, 2, YC], dt.bfloat16, tag="ysb", name="ysb")
                    for j in range(2):
                        ch = 2 * hy + j
                        o0 = ch * YC
                        py = pA.tile([P, YC], dt.float32, tag="pb", name="pb")
                        for kk in range(4):
                            nc.tensor.matmul(py, att[:, kk, :],
                                             wout_sb[kk][:, o0:o0 + YC],
                                             start=(kk == 0), stop=(kk == 3))
                        if ch % 2 == 0:
                            nc.vector.tensor_copy(ysb[:, j, :], py)
                        else:
                            nc.scalar.copy(ysb[:, j, :], py)
                    nc.scalar.dma_start(
                        out=y_d[b * P:(b + 1) * P, hy * 2 * YC:(hy + 1) * 2 * YC],
                        in_=ysb)

            for half in range(2):
                kv_half(half)
                q_half(half)
                for j in range(4):
                    attention(half * 4 + j)

    nc.compile()
    return nc


# ----------------------------------------------------------------------------
# public entry
# ----------------------------------------------------------------------------

LAST_RESULTS = None


def kernel(x, norm_scale, qkv_w, qkv_b, out_w, out_b, sinks):
    global LAST_RESULTS
    x = np.asarray(x, dtype=np.float32)
    norm_scale = np.asarray(norm_scale, dtype=np.float32)
    qkv_w = np.asarray(qkv_w, dtype=np.float32)
    qkv_b = np.asarray(qkv_b, dtype=np.float32)
    out_w = np.asarray(out_w, dtype=np.float32)
    out_b = np.asarray(out_b, dtype=np.float32)
    sinks = np.asarray(sinks, dtype=np.float32)

    if "nc" not in _CACHE:
        _CACHE["nc"] = build_nc()
    nc = _CACHE["nc"]

    in_maps = [
        _prep_core_inputs(c, x, norm_scale, qkv_w, qkv_b, out_w, sinks)
        for c in range(NCORES)
    ]
    import os
    tmpdir = os.environ.get("BASS_TMPDIR") or None
    res = run_bass_kernel_spmd(nc, in_maps, core_ids=list(range(NCORES)),
                               tmpdir=tmpdir)
    LAST_RESULTS = res
    y = np.zeros((T, HIDDEN), dtype=np.float64)
    for c in range(NCORES):
        y += res.results[c]["y"].astype(np.float64)
    y += out_b.astype(np.float64)[None, :]
    return y.astype(np.float32)


# revision 21
# speedup vs baseline: 1.4952x; 1.4952x over previous
"""Trainium2 Bass kernel: sparse (sliding-window) attention block, v3.

Full module per reference:
  RMSNorm -> fused QKV (5120x2880) -> YaRN RoPE -> GQA sliding-window(128)
  causal attention with learned sink logit -> out projection (2880x4096).

Sharding: tensor-parallel over heads across 8 cores. Core c owns q-heads
[8c, 8c+8) and kv-head c. Each core emits a partial [1024, 2880] bf16
output; the host sums the partials (f64) and adds out_b.

v3 = v1 attention backend (q-on-partitions AV with ones-column
denominators, tiny reciprocals, PE transposes for the out-proj lhsT)
plus the v2 scheduling wins:
  - both activation tables (Ln, Exp) prefetched with dummy ops at t0; no
    Square activation anywhere (x^2 via vector bf16 muls) -> no table
    swaps, no startup stall.
  - half-granularity pipeline: kv+q projection and rope for tokens
    [0,512) are followed immediately by attention+out-proj of tiles 0-3
    while the second half's QKV matmuls still run.
  - sm_scale folded into the Exp activation scale -> q and k share one
    plain cos/sin table pair (half the table DMA/SBUF).
  - masks multiplicative {0,1} bf16 applied to the exp'd probabilities.
  - y written bf16 in [128, 960] chunks DMA'd from the scalar queue.
"""

import math
import sys

import numpy as np

try:
    import concourse.bass as bass
except ImportError:  # pragma: no cover
    sys.path.insert(0, "/opt/trn_rl_repo")
    import concourse.bass as bass

import concourse.bacc as bacc
import concourse.tile as tile
from concourse import mybir
from concourse.masks import make_identity
from concourse.bass_utils import run_bass_kernel_spmd

import ml_dtypes

BF16 = ml_dtypes.bfloat16

T = 1024
HIDDEN = 2880
HD = 64
NH = 64
NKV = 8
SW = 128
NCORES = 8
HPC = NH // NCORES          # q heads per core = 8
QKV_DIM = HD * (NH + 2 * NKV)
SM_SCALE = 1.0 / math.sqrt(HD)

P = 128
KT = (HIDDEN + P - 1) // P   # 23 k-tiles over hidden (zero-padded to 2944)
KPAD = KT * P
NT = 5                       # qkv n-tiles of 128 (4 q-tiles + 1 kv-tile)
MT = T // P                  # 8 token tiles
AW = HD + 1                  # AV width: 64 v dims + denominator column

dt = mybir.dt

_CACHE = {}


# ----------------------------------------------------------------------------
# host-side helpers
# ----------------------------------------------------------------------------

def _rope_cos_sin(num_tokens):
    base = 150000.0
    scaling = 32.0
    init_ctx = 4096.0
    ntk_alpha = 1.0
    ntk_beta = 32.0
    d_half = HD / 2
    freq = base ** (np.arange(0, HD, 2, dtype=np.float32) / HD)
    concentration = 0.1 * math.log(scaling) + 1.0
    low = d_half * math.log(init_ctx / (ntk_beta * 2 * math.pi)) / math.log(base)
    high = d_half * math.log(init_ctx / (ntk_alpha * 2 * math.pi)) / math.log(base)
    interpolation = 1.0 / (scaling * freq)
    extrapolation = 1.0 / freq
    ramp = (np.arange(int(d_half), dtype=np.float32) - low) / (high - low)
    m = 1.0 - np.clip(ramp, 0.0, 1.0)
    inv_freq = interpolation * (1.0 - m) + extrapolation * m
    t = np.arange(num_tokens, dtype=np.float32)
    freqs = t[:, None] * inv_freq[None, :]
    cos = (np.cos(freqs) * concentration).astype(np.float32)
    sin = (np.sin(freqs) * concentration).astype(np.float32)
    return cos, sin  # [T, 32]


def _host_tables():
    """Plain (unscaled) replicated rope tables with the swap sign folded
    into sin: rope(u)[p] = u[p]*cos[p] + u[p^32]*sin_alt[p]."""
    cos, sin = _rope_cos_sin(T)  # [1024, 32]
    sgn = np.repeat([-1.0, 1.0], 32)[:, None].astype(np.float32)
    sgn = np.tile(sgn, (2, 1))  # [128, 1]
    cos_t = np.tile(cos.T, (4, 1)).astype(np.float32)          # [128, 1024]
    sin_t = (np.tile(sin.T, (4, 1)) * sgn).astype(np.float32)  # [128, 1024]
    return cos_t, sin_t


def _host_masks01():
    j = np.arange(P)[:, None]   # kt row (partition)
    i = np.arange(P)[None, :]   # q col (free)
    mask_prev = (j > i).astype(np.float32)    # dist in [1,127]
    mask_self = (j <= i).astype(np.float32)   # dist in [0,127]
    return np.concatenate([mask_prev, mask_self], axis=1).astype(BF16)


def _prep_core_inputs(core, x, norm_scale, qkv_w, qkv_b, out_w, sinks):
    q_end = NH * HD
    k_end = q_end + NKV * HD

    qrows = np.arange(core * HPC * HD, (core + 1) * HPC * HD)
    krows = np.arange(q_end + core * HD, q_end + (core + 1) * HD)
    vrows = np.arange(k_end + core * HD, k_end + (core + 1) * HD)
    # kv n-tile: v in partitions 0:64, k in 64:128
    rows = np.concatenate([qrows, vrows, krows])  # [640]

    wshard = (qkv_w[rows, :] * norm_scale[None, :]).astype(np.float32)
    bshard = qkv_b[rows].astype(np.float32)  # [640]

    # lhsT tiles: wq[n, kp, kt*128 + nc] = wshard[n*128 + nc, kt*128 + kp]
    wq = np.zeros((NT, P, KPAD), dtype=BF16)
    for n in range(NT):
        blk = wshard[n * P:(n + 1) * P, :]  # [128 n, 2880 k]
        for ki in range(KT):
            k0 = ki * P
            ksz = min(P, HIDDEN - k0)
            wq[n, :ksz, ki * P:ki * P + P] = blk[:, k0:k0 + ksz].T.astype(BF16)

    cols = np.arange(core * HPC * HD, (core + 1) * HPC * HD)
    wo = out_w[:, cols].T.astype(np.float32)  # [512 hd, 2880 H]
    wout = wo.reshape(4, P, HIDDEN).astype(BF16)

    bqkv = bshard.reshape(NT, P).T.copy().astype(np.float32)  # [128, 5]

    cos_t, sin_t = _host_tables()  # [128, 1024] f32 each

    xt = np.zeros((KPAD, T), dtype=BF16)
    xt[:HIDDEN] = x.T.astype(BF16)

    esink = np.exp(sinks[core * HPC:(core + 1) * HPC].astype(np.float64))
    esink = np.broadcast_to(esink.astype(np.float32), (P, HPC)).copy()

    return {
        "xt": xt,                               # [2944, 1024] bf16
        "wq": wq,                               # [5, 128, 2944] bf16
        "wout": wout,                           # [4, 128, 2880] bf16
        "bqkv": bqkv,                           # [128, 5] f32
        "cos_t": cos_t, "sin_t": sin_t,         # [128, 1024] f32
        "mask": _host_masks01(),                # [128, 256] bf16
        "esink": esink,                         # [128, 8] f32
    }


# ----------------------------------------------------------------------------
# device kernel (Tile)
# ----------------------------------------------------------------------------

def build_nc():
    nc = bacc.Bacc("TRN2", target_bir_lowering=False, debug=False)

    xt_d = nc.dram_tensor("xt", [KPAD, T], dt.bfloat16, kind="ExternalInput").ap()
    wq_d = nc.dram_tensor("wq", [NT, P, KPAD], dt.bfloat16, kind="ExternalInput").ap()
    wout_d = nc.dram_tensor("wout", [4, P, HIDDEN], dt.bfloat16, kind="ExternalInput").ap()
    bqkv_d = nc.dram_tensor("bqkv", [P, NT], dt.float32, kind="ExternalInput").ap()
    cos_d = nc.dram_tensor("cos_t", [P, T], dt.float32, kind="ExternalInput").ap()
    sin_d = nc.dram_tensor("sin_t", [P, T], dt.float32, kind="ExternalInput").ap()
    mask_d = nc.dram_tensor("mask", [P, 2 * P], dt.bfloat16, kind="ExternalInput").ap()
    esink_d = nc.dram_tensor("esink", [P, HPC], dt.float32, kind="ExternalInput").ap()
    y_d = nc.dram_tensor("y", [T, HIDDEN], dt.bfloat16, kind="ExternalOutput").ap()

    YC = 480

    def bcast_mid(ap2d, n):
        """[P, F] -> [P, n, F] with a 0-step middle dim (free broadcast)."""
        return bass.AP(tensor=ap2d.tensor, offset=ap2d.offset,
                       ap=[ap2d.ap[0], [0, n]] + list(ap2d.ap[1:]))

    with tile.TileContext(nc) as tc:
        with (
            tc.tile_pool(name="const", bufs=1) as const,
            tc.tile_pool(name="res", bufs=1) as res,
            tc.tile_pool(name="qkvp", bufs=2) as qkvp,
            tc.tile_pool(name="xsqp", bufs=2) as xsqp,
            tc.tile_pool(name="ropep", bufs=2) as ropep,
            tc.tile_pool(name="ptp", bufs=2) as ptp,
            tc.tile_pool(name="smallp", bufs=2) as smallp,
            tc.tile_pool(name="anp", bufs=2) as anp,
            tc.tile_pool(name="attp", bufs=2) as attp,
            tc.tile_pool(name="ysbp", bufs=3) as ysbp,
            tc.tile_pool(name="pA", bufs=2, space="PSUM") as pA,
            tc.tile_pool(name="pY", bufs=2, space="PSUM") as pY,
            tc.tile_pool(name="pS", bufs=2, space="PSUM") as pS,
        ):
            # ---- constants ----
            zbias = const.tile([P, 1], dt.float32, tag="zbias", name="zbias")
            nc.vector.memset(zbias, 0.0)
            eps_t = const.tile([1, 1], dt.float32, tag="eps", name="eps_t")
            nc.vector.memset(eps_t, 1e-5)
            ones = const.tile([P, 1], dt.bfloat16, tag="ones", name="ones")
            nc.vector.memset(ones, 1.0)
            identb = const.tile([P, P], dt.bfloat16, tag="identb", name="identb")
            make_identity(nc, identb)
            # prefetch both activation tables (Ln, Exp) with dummy ops
            dmy = const.tile([1, 2], dt.float32, tag="dmy", name="dmy")
            nc.scalar.activation(dmy[:, 0:1], eps_t,
                                 mybir.ActivationFunctionType.Ln,
                                 bias=eps_t)
            nc.scalar.activation(dmy[:, 1:2], eps_t,
                                 mybir.ActivationFunctionType.Exp,
                                 bias=zbias[0:1, :])

            # ---- DMA issue (sync queue; y outputs go on the scalar queue) ----
            wq_sb = [res.tile([P, KPAD], dt.bfloat16, tag=f"wq{n}", name=f"wq{n}")
                     for n in range(NT)]
            xt_sb = res.tile([P, KT, T], dt.bfloat16, tag="xt", name="xt")
            wout_sb = [res.tile([P, HIDDEN], dt.bfloat16, tag=f"wout{kk}",
                                name=f"wout{kk}") for kk in range(4)]
            cos_sb = const.tile([P, T], dt.float32, tag="cos", name="cos_sb")
            sin_sb = const.tile([P, T], dt.float32, tag="sin", name="sin_sb")
            mask_sb = const.tile([P, 2 * P], dt.bfloat16, tag="mask", name="mask_sb")
            esink_sb = const.tile([P, HPC], dt.float32, tag="esink", name="esink_sb")
            bqkv_sb = const.tile([P, NT], dt.float32, tag="bqkv", name="bqkv_sb")

            HK = KPAD // 2
            HO = HIDDEN // 2
            HT = T // 2

            def dma(out, in_):
                nc.sync.dma_start(out=out, in_=in_)

            # kv weights first; xt interleaved so the qkv4 chain can pace
            dma(wq_sb[4][:, :HK], wq_d[4, :, :HK])
            for ki in range(0, 4):
                dma(xt_sb[:, ki, :], xt_d[ki * P:(ki + 1) * P, :])
            dma(wq_sb[4][:, HK:], wq_d[4, :, HK:])
            for ki in range(4, 8):
                dma(xt_sb[:, ki, :], xt_d[ki * P:(ki + 1) * P, :])
            dma(wq_sb[0][:, :HK], wq_d[0, :, :HK])
            dma(wq_sb[0][:, HK:], wq_d[0, :, HK:])
            for ki in range(8, 12):
                dma(xt_sb[:, ki, :], xt_d[ki * P:(ki + 1) * P, :])
            dma(cos_sb[:, :HT], cos_d[:, :HT])
            dma(sin_sb[:, :HT], sin_d[:, :HT])
            dma(cos_sb[:, HT:], cos_d[:, HT:])
            dma(sin_sb[:, HT:], sin_d[:, HT:])
            dma(mask_sb, mask_d)
            dma(esink_sb, esink_d)
            dma(bqkv_sb, bqkv_d)
            dma(wq_sb[1][:, :HK], wq_d[1, :, :HK])
            dma(wq_sb[1][:, HK:], wq_d[1, :, HK:])
            for ki in range(12, 16):
                dma(xt_sb[:, ki, :], xt_d[ki * P:(ki + 1) * P, :])
            dma(wout_sb[0][:, :HO], wout_d[0, :, :HO])
            dma(wout_sb[0][:, HO:], wout_d[0, :, HO:])
            dma(wq_sb[2][:, :HK], wq_d[2, :, :HK])
            dma(wq_sb[2][:, HK:], wq_d[2, :, HK:])
            for ki in range(16, 20):
                dma(xt_sb[:, ki, :], xt_d[ki * P:(ki + 1) * P, :])
            for kk in range(1, 4):
                dma(wout_sb[kk][:, :HO], wout_d[kk, :, :HO])
                dma(wout_sb[kk][:, HO:], wout_d[kk, :, HO:])
            dma(wq_sb[3][:, :HK], wq_d[3, :, :HK])
            dma(wq_sb[3][:, HK:], wq_d[3, :, HK:])
            for ki in range(20, KT):
                dma(xt_sb[:, ki, :], xt_d[ki * P:(ki + 1) * P, :])

            # ---- sum of squares: vector bf16 squares + PE ones-reduction ----
            psum_ssq = [pY.tile([1, 512], dt.float32, tag="py", name=f"ssq{h}")
                        for h in range(2)]
            for ki in range(KT):
                xsq = xsqp.tile([P, T], dt.bfloat16, tag="xsq", name="xsq")
                nc.vector.tensor_mul(xsq, xt_sb[:, ki, :], xt_sb[:, ki, :])
                for half in range(2):
                    nc.tensor.matmul(
                        psum_ssq[half],
                        ones,
                        xsq[:, half * 512:half * 512 + 512],
                        start=(ki == 0), stop=(ki == KT - 1),
                    )

            # rsq_b = exp(-0.5*ln(ssq/H + eps)) broadcast to 128 partitions
            lnm = res.tile([1, T], dt.float32, tag="lnm", name="lnm")
            for half in range(2):
                nc.scalar.activation(lnm[:, half * 512:half * 512 + 512],
                                     psum_ssq[half],
                                     mybir.ActivationFunctionType.Ln,
                                     bias=eps_t, scale=1.0 / HIDDEN)
            rinv = res.tile([1, T], dt.float32, tag="rinv", name="rinv")
            nc.scalar.activation(rinv, lnm, mybir.ActivationFunctionType.Exp,
                                 bias=zbias[0:1, :], scale=-0.5)
            rsq_b = res.tile([P, T], dt.float32, tag="rsq", name="rsq_b")
            nc.gpsimd.partition_broadcast(rsq_b, rinv)

            # ---- qkv projection (scaled+biased, bf16) ----
            def qkv_tile(n, half, dst):
                """dst <- (W x)*rsq + b for columns [half*512, ...+512)."""
                c0 = half * 512
                pq = pA.tile([P, 512], dt.float32, tag="pb", name="pb")
                for ki in range(KT):
                    nc.tensor.matmul(
                        pq,
                        wq_sb[n][:, ki * P:ki * P + P],
                        xt_sb[:, ki, c0:c0 + 512],
                        start=(ki == 0), stop=(ki == KT - 1),
                    )
                nc.vector.tensor_mul(dst, pq, rsq_b[:, c0:c0 + 512])
                nc.vector.tensor_scalar_add(dst, dst, bqkv_sb[:, n:n + 1])

            qra = res.tile([64, HPC, T], dt.bfloat16, tag="qra", name="qra")
            krope = res.tile([64, T], dt.bfloat16, tag="krope", name="krope")
            # all 8 token-major v tiles in one buffer; ones column prefilled
            vtok = res.tile([P, MT, AW], dt.bfloat16, tag="vtok", name="vtok")
            nc.vector.memset(vtok[:, :, HD:HD + 1], 1.0)

            qkvT4 = res.tile([P, T], dt.bfloat16, tag="qkvT4", name="qkvT4")

            def kv_phase(half):
                c0 = half * 512
                qkv_tile(4, half, qkvT4[:, c0:c0 + 512])
                for j in range(4):
                    b = half * 4 + j
                    pv = pS.tile([P, HD], dt.bfloat16, tag="sc", name="pv")
                    nc.tensor.transpose(pv, qkvT4[0:64, b * P:(b + 1) * P],
                                        identb[:64, :64])
                    nc.vector.tensor_copy(vtok[:, b, 0:HD], pv)
                # k rope (rows 64:128) at half width
                kswp = ropep.tile([P, 512], dt.bfloat16, tag="hswp", name="kswp")
                for a in (64, 96):
                    nc.vector.tensor_copy(kswp[a:a + 32, :],
                                          qkvT4[a ^ 32:(a ^ 32) + 32,
                                                c0:c0 + 512])
                ktc = ropep.tile([P, 512], dt.float32, tag="htc", name="ktc")
                nc.vector.tensor_mul(ktc[64:128, :], qkvT4[64:128, c0:c0 + 512],
                                     cos_sb[64:128, c0:c0 + 512])
                kts = ropep.tile([P, 512], dt.float32, tag="hts", name="kts")
                nc.vector.tensor_mul(kts[64:128, :], kswp[64:128, :],
                                     sin_sb[64:128, c0:c0 + 512])
                nc.vector.tensor_add(krope[:, c0:c0 + 512], ktc[64:128, :],
                                     kts[64:128, :])

            def q_phase(half):
                c0 = half * 512
                for n in range(4):
                    qkvt = qkvp.tile([P, 512], dt.bfloat16, tag="qkvT",
                                     name=f"qkvt{n}")
                    qkv_tile(n, half, qkvt)
                    # rope at half width on the fly
                    swp = ropep.tile([P, 512], dt.bfloat16, tag="hswp", name="hswp")
                    for a in range(0, P, 32):
                        nc.vector.tensor_copy(swp[a:a + 32, :],
                                              qkvt[a ^ 32:(a ^ 32) + 32, :])
                    tc_ = ropep.tile([P, 512], dt.float32, tag="htc", name="htc")
                    nc.vector.tensor_mul(tc_, qkvt, cos_sb[:, c0:c0 + 512])
                    ts_ = ropep.tile([P, 512], dt.float32, tag="hts", name="hts")
                    nc.vector.tensor_mul(ts_, swp, sin_sb[:, c0:c0 + 512])
                    for i in range(2):
                        b0 = 64 * i
                        nc.vector.tensor_add(qra[:, 2 * n + i, c0:c0 + 512],
                                             tc_[b0:b0 + 64, :],
                                             ts_[b0:b0 + 64, :])

            # ---- attention + out-projection for one token tile ----
            def attention(b):
                pt = ptp.tile([P, 2, HPC, P], dt.bfloat16, tag="pt", name="pt")
                kts = [(0, b - 1), (1, b)] if b > 0 else [(1, b)]
                for s, kt in kts:
                    for g in range(2):
                        ps = pS.tile([P, 4, P], dt.float32, tag="sc", name="sc")
                        nc.tensor.matmul(
                            ps,
                            krope[:, kt * P:(kt + 1) * P],
                            qra[:, 4 * g:4 * g + 4, b * P:(b + 1) * P],
                            start=True, stop=True)
                        nc.scalar.activation(pt[:, s, 4 * g:4 * g + 4, :], ps,
                                             mybir.ActivationFunctionType.Exp,
                                             bias=zbias, scale=SM_SCALE)
                    m0 = 0 if s == 0 else P
                    nc.vector.tensor_mul(pt[:, s], pt[:, s],
                                         bcast_mid(mask_sb[:, m0:m0 + P], HPC))

                # AV with ones column -> denominators in column 64
                rec8 = smallp.tile([P, HPC], dt.float32, tag="rec8", name="rec8")
                an = anp.tile([P, HPC, HD], dt.bfloat16, tag="an", name="an")
                for g in range(2):
                    pg = pS.tile([P, 4, AW], dt.float32, tag="sc", name="pg")
                    for j in range(4):
                        h = 4 * g + j
                        for idx, (s, kt) in enumerate(kts):
                            nc.tensor.matmul(pg[:, j, :], pt[:, s, h, :],
                                             vtok[:, kt, :],
                                             start=(idx == 0),
                                             stop=(idx == len(kts) - 1))
                    g0 = 4 * g
                    nc.vector.tensor_add(rec8[:, g0:g0 + 4],
                                         pg[:, :, HD:HD + 1],
                                         esink_sb[:, g0:g0 + 4])
                    nc.vector.reciprocal(rec8[:, g0:g0 + 4], rec8[:, g0:g0 + 4])
                    rec3 = bass.AP(tensor=rec8.tensor,
                                   offset=rec8[:, g0:g0 + 4].offset,
                                   ap=[rec8.ap[0], [1, 4], [0, HD]])
                    nc.vector.tensor_mul(an[:, g0:g0 + 4, :], pg[:, :, 0:HD],
                                         rec3)

                # transpose to out-proj lhsT layout [128 hd, 128 tok]
                att = attp.tile([P, 4, P], dt.bfloat16, tag="att", name="att")
                a2 = an.rearrange("p a b -> p (a b)")
                for kk in range(4):
                    pat = pS.tile([P, P], dt.bfloat16, tag="sc", name="pat")
                    nc.tensor.transpose(pat, a2[:, kk * P:(kk + 1) * P], identb)
                    if kk % 2 == 0:
                        nc.vector.tensor_copy(att[:, kk, :], pat)
                    else:
                        nc.scalar.copy(att[:, kk, :], pat)

                # out projection in 960-wide pairs, bf16 out, DMA on scalar q
                for hy in range(3):
                    pyt = pY.tile([P, 2, 512], dt.float32, tag="py", name="py")
                    for j in range(2):
                        o0 = (2 * hy + j) * YC
                        for kk in range(4):
                            nc.tensor.matmul(pyt[:, j, 0:YC], att[:, kk, :],
                                             wout_sb[kk][:, o0:o0 + YC],
                                             start=(kk == 0), stop=(kk == 3))
                    ysb = ysbp.tile([P, 2, YC], dt.bfloat16, tag="ysb", name="ysb")
                    if hy % 2 == 0:
                        nc.vector.tensor_copy(ysb, pyt[:, :, 0:YC])
                    else:
                        nc.scalar.copy(ysb, pyt[:, :, 0:YC])
                    nc.scalar.dma_start(
                        out=y_d[b * P:(b + 1) * P, hy * 2 * YC:(hy + 1) * 2 * YC],
                        in_=ysb)

            for half in range(2):
                kv_phase(half)
                q_phase(half)
                for j in range(4):
                    attention(half * 4 + j)

    nc.compile()
    return nc


# ----------------------------------------------------------------------------
# public entry
# ----------------------------------------------------------------------------

LAST_RESULTS = None


def kernel(x, norm_scale, qkv_w, qkv_b, out_w, out_b, sinks):
    global LAST_RESULTS
    x = np.asarray(x, dtype=np.float32)
    norm_scale = np.asarray(norm_scale, dtype=np.float32)
    qkv_w = np.asarray(qkv_w, dtype=np.float32)
    qkv_b = np.asarray(qkv_b, dtype=np.float32)
    out_w = np.asarray(out_w, dtype=np.float32)
    out_b = np.asarray(out_b, dtype=np.float32)
    sinks = np.asarray(sinks, dtype=np.float32)

    if "nc" not in _CACHE:
        _CACHE["nc"] = build_nc()
    nc = _CACHE["nc"]

    in_maps = [
        _prep_core_inputs(c, x, norm_scale, qkv_w, qkv_b, out_w, sinks)
        for c in range(NCORES)
    ]
    import os
    tmpdir = os.environ.get("BASS_TMPDIR") or None
    res = run_bass_kernel_spmd(nc, in_maps, core_ids=list(range(NCORES)),
                               tmpdir=tmpdir)
    LAST_RESULTS = res
    y = np.zeros((T, HIDDEN), dtype=np.float64)
    for c in range(NCORES):
        y += res.results[c]["y"].astype(np.float64)
    y += out_b.astype(np.float64)[None, :]
    return y.astype(np.float32)


# revision 27
# speedup vs baseline: 1.6239x; 1.0861x over previous
"""Trainium2 Bass kernel: sparse (sliding-window) attention block, v3.

Full module per reference:
  RMSNorm -> fused QKV (5120x2880) -> YaRN RoPE -> GQA sliding-window(128)
  causal attention with learned sink logit -> out projection (2880x4096).

Sharding: tensor-parallel over heads across 8 cores. Core c owns q-heads
[8c, 8c+8) and kv-head c. Each core emits a partial [1024, 2880] bf16
output; the host sums the partials (f64) and adds out_b.

v3 = v1 attention backend (q-on-partitions AV with ones-column
denominators, tiny reciprocals, PE transposes for the out-proj lhsT)
plus the v2 scheduling wins:
  - both activation tables (Ln, Exp) prefetched with dummy ops at t0; no
    Square activation anywhere (x^2 via vector bf16 muls) -> no table
    swaps, no startup stall.
  - half-granularity pipeline: kv+q projection and rope for tokens
    [0,512) are followed immediately by attention+out-proj of tiles 0-3
    while the second half's QKV matmuls still run.
  - sm_scale folded into the Exp activation scale -> q and k share one
    plain cos/sin table pair (half the table DMA/SBUF).
  - masks multiplicative {0,1} bf16 applied to the exp'd probabilities.
  - y written bf16 in [128, 960] chunks DMA'd from the scalar queue.
"""

import math
import sys

import numpy as np

try:
    import concourse.bass as bass
except ImportError:  # pragma: no cover
    sys.path.insert(0, "/opt/trn_rl_repo")
    import concourse.bass as bass

import concourse.bacc as bacc
import concourse.tile as tile
from concourse import mybir
from concourse.masks import make_identity
from concourse.bass_utils import run_bass_kernel_spmd

import ml_dtypes

BF16 = ml_dtypes.bfloat16

T = 1024
HIDDEN = 2880
HD = 64
NH = 64
NKV = 8
SW = 128
NCORES = 8
HPC = NH // NCORES          # q heads per core = 8
QKV_DIM = HD * (NH + 2 * NKV)
SM_SCALE = 1.0 / math.sqrt(HD)

P = 128
KT = (HIDDEN + P - 1) // P   # 23 k-tiles over hidden (zero-padded to 2944)
KPAD = KT * P
NT = 5                       # qkv n-tiles of 128 (4 q-tiles + 1 kv-tile)
MT = T // P                  # 8 token tiles
AW = HD + 1                  # AV width: 64 v dims + denominator column

dt = mybir.dt

_CACHE = {}


# ----------------------------------------------------------------------------
# host-side helpers
# ----------------------------------------------------------------------------

def _rope_cos_sin(num_tokens):
    base = 150000.0
    scaling = 32.0
    init_ctx = 4096.0
    ntk_alpha = 1.0
    ntk_beta = 32.0
    d_half = HD / 2
    freq = base ** (np.arange(0, HD, 2, dtype=np.float32) / HD)
    concentration = 0.1 * math.log(scaling) + 1.0
    low = d_half * math.log(init_ctx / (ntk_beta * 2 * math.pi)) / math.log(base)
    high = d_half * math.log(init_ctx / (ntk_alpha * 2 * math.pi)) / math.log(base)
    interpolation = 1.0 / (scaling * freq)
    extrapolation = 1.0 / freq
    ramp = (np.arange(int(d_half), dtype=np.float32) - low) / (high - low)
    m = 1.0 - np.clip(ramp, 0.0, 1.0)
    inv_freq = interpolation * (1.0 - m) + extrapolation * m
    t = np.arange(num_tokens, dtype=np.float32)
    freqs = t[:, None] * inv_freq[None, :]
    cos = (np.cos(freqs) * concentration).astype(np.float32)
    sin = (np.sin(freqs) * concentration).astype(np.float32)
    return cos, sin  # [T, 32]


def _host_tables():
    """Plain (unscaled) replicated rope tables with the swap sign folded
    into sin: rope(u)[p] = u[p]*cos[p] + u[p^32]*sin_alt[p]."""
    cos, sin = _rope_cos_sin(T)  # [1024, 32]
    sgn = np.repeat([-1.0, 1.0], 32)[:, None].astype(np.float32)
    sgn = np.tile(sgn, (2, 1))  # [128, 1]
    cos_t = np.tile(cos.T, (4, 1)).astype(np.float32)          # [128, 1024]
    sin_t = (np.tile(sin.T, (4, 1)) * sgn).astype(np.float32)  # [128, 1024]
    return cos_t, sin_t


def _host_masks01():
    j = np.arange(P)[:, None]   # kt row (partition)
    i = np.arange(P)[None, :]   # q col (free)
    mask_prev = (j > i).astype(np.float32)    # dist in [1,127]
    mask_self = (j <= i).astype(np.float32)   # dist in [0,127]
    return np.concatenate([mask_prev, mask_self], axis=1).astype(BF16)


def _prep_core_inputs(core, x, norm_scale, qkv_w, qkv_b, out_w, sinks):
    q_end = NH * HD
    k_end = q_end + NKV * HD

    qrows = np.arange(core * HPC * HD, (core + 1) * HPC * HD)
    krows = np.arange(q_end + core * HD, q_end + (core + 1) * HD)
    vrows = np.arange(k_end + core * HD, k_end + (core + 1) * HD)
    # kv n-tile: v in partitions 0:64, k in 64:128
    rows = np.concatenate([qrows, vrows, krows])  # [640]

    wshard = (qkv_w[rows, :] * norm_scale[None, :]).astype(np.float32)
    bshard = qkv_b[rows].astype(np.float32)  # [640]

    # lhsT tiles: wq[n, kp, kt*128 + nc] = wshard[n*128 + nc, kt*128 + kp]
    wq = np.zeros((NT, P, KPAD), dtype=BF16)
    for n in range(NT):
        blk = wshard[n * P:(n + 1) * P, :]  # [128 n, 2880 k]
        for ki in range(KT):
            k0 = ki * P
            ksz = min(P, HIDDEN - k0)
            wq[n, :ksz, ki * P:ki * P + P] = blk[:, k0:k0 + ksz].T.astype(BF16)

    cols = np.arange(core * HPC * HD, (core + 1) * HPC * HD)
    wo = out_w[:, cols].T.astype(np.float32)  # [512 hd, 2880 H]
    wout = wo.reshape(4, P, HIDDEN).astype(BF16)

    bqkv = bshard.reshape(NT, P).T.copy().astype(np.float32)  # [128, 5]

    cos_t, sin_t = _host_tables()  # [128, 1024] f32 each

    xt = np.zeros((KPAD, T), dtype=BF16)
    xt[:HIDDEN] = x.T.astype(BF16)

    esink = np.exp(sinks[core * HPC:(core + 1) * HPC].astype(np.float64))
    esink = np.broadcast_to(esink.astype(np.float32), (P, HPC)).copy()

    return {
        "xt": xt,                               # [2944, 1024] bf16
        "wq": wq,                               # [5, 128, 2944] bf16
        "wout": wout,                           # [4, 128, 2880] bf16
        "bqkv": bqkv,                           # [128, 5] f32
        "cos_t": cos_t, "sin_t": sin_t,         # [128, 1024] f32
        "mask": _host_masks01(),                # [128, 256] bf16
        "esink": esink,                         # [128, 8] f32
    }


# ----------------------------------------------------------------------------
# device kernel (Tile)
# ----------------------------------------------------------------------------

def build_nc():
    nc = bacc.Bacc("TRN2", target_bir_lowering=False, debug=False)

    xt_d = nc.dram_tensor("xt", [KPAD, T], dt.bfloat16, kind="ExternalInput").ap()
    wq_d = nc.dram_tensor("wq", [NT, P, KPAD], dt.bfloat16, kind="ExternalInput").ap()
    wout_d = nc.dram_tensor("wout", [4, P, HIDDEN], dt.bfloat16, kind="ExternalInput").ap()
    bqkv_d = nc.dram_tensor("bqkv", [P, NT], dt.float32, kind="ExternalInput").ap()
    cos_d = nc.dram_tensor("cos_t", [P, T], dt.float32, kind="ExternalInput").ap()
    sin_d = nc.dram_tensor("sin_t", [P, T], dt.float32, kind="ExternalInput").ap()
    mask_d = nc.dram_tensor("mask", [P, 2 * P], dt.bfloat16, kind="ExternalInput").ap()
    esink_d = nc.dram_tensor("esink", [P, HPC], dt.float32, kind="ExternalInput").ap()
    y_d = nc.dram_tensor("y", [T, HIDDEN], dt.bfloat16, kind="ExternalOutput").ap()

    YC = 480

    def bcast_mid(ap2d, n):
        """[P, F] -> [P, n, F] with a 0-step middle dim (free broadcast)."""
        return bass.AP(tensor=ap2d.tensor, offset=ap2d.offset,
                       ap=[ap2d.ap[0], [0, n]] + list(ap2d.ap[1:]))

    with tile.TileContext(nc) as tc:
        with (
            tc.tile_pool(name="const", bufs=1) as const,
            tc.tile_pool(name="res", bufs=1) as res,
            tc.tile_pool(name="qkvp", bufs=2) as qkvp,
            tc.tile_pool(name="xsqp", bufs=2) as xsqp,
            tc.tile_pool(name="ropep", bufs=2) as ropep,
            tc.tile_pool(name="ptp", bufs=2) as ptp,
            tc.tile_pool(name="smallp", bufs=2) as smallp,
            tc.tile_pool(name="anp", bufs=2) as anp,
            tc.tile_pool(name="attp", bufs=2) as attp,
            tc.tile_pool(name="ysbp", bufs=3) as ysbp,
            tc.tile_pool(name="pA", bufs=2, space="PSUM") as pA,
            tc.tile_pool(name="pY", bufs=3, space="PSUM") as pY,
            tc.tile_pool(name="pS", bufs=3, space="PSUM") as pS,
        ):
            # ---- constants ----
            zbias = const.tile([P, 1], dt.float32, tag="zbias", name="zbias")
            nc.vector.memset(zbias, 0.0)
            eps_t = const.tile([1, 1], dt.float32, tag="eps", name="eps_t")
            nc.vector.memset(eps_t, 1e-5)
            ones = const.tile([P, 1], dt.bfloat16, tag="ones", name="ones")
            nc.vector.memset(ones, 1.0)
            identb = const.tile([P, P], dt.bfloat16, tag="identb", name="identb")
            make_identity(nc, identb)
            # prefetch both activation tables (Ln, Exp) with dummy ops
            dmy = const.tile([1, 2], dt.float32, tag="dmy", name="dmy")
            nc.scalar.activation(dmy[:, 0:1], eps_t,
                                 mybir.ActivationFunctionType.Ln,
                                 bias=eps_t)
            nc.scalar.activation(dmy[:, 1:2], eps_t,
                                 mybir.ActivationFunctionType.Exp,
                                 bias=zbias[0:1, :])

            # ---- DMA issue (sync queue; y outputs go on the scalar queue) ----
            wq_sb = [res.tile([P, KPAD], dt.bfloat16, tag=f"wq{n}", name=f"wq{n}")
                     for n in range(NT)]
            xt_sb = res.tile([P, KT, T], dt.bfloat16, tag="xt", name="xt")
            wout_sb = [res.tile([P, HIDDEN], dt.bfloat16, tag=f"wout{kk}",
                                name=f"wout{kk}") for kk in range(4)]
            cos_sb = const.tile([P, T], dt.float32, tag="cos", name="cos_sb")
            sin_sb = const.tile([P, T], dt.float32, tag="sin", name="sin_sb")
            mask_sb = const.tile([P, 2 * P], dt.bfloat16, tag="mask", name="mask_sb")
            esink_sb = const.tile([P, HPC], dt.float32, tag="esink", name="esink_sb")
            bqkv_sb = const.tile([P, NT], dt.float32, tag="bqkv", name="bqkv_sb")

            HK = KPAD // 2
            HO = HIDDEN // 2
            HT = T // 2

            def dma(out, in_):
                nc.sync.dma_start(out=out, in_=in_)

            # xt first (rsq path is the long pole); first tile split for an
            # early pipeline start; weights interleaved in need order
            dma(xt_sb[:, 0, 0:512], xt_d[0:P, 0:512])
            dma(xt_sb[:, 0, 512:], xt_d[0:P, 512:])
            dma(wq_sb[4][:, :HK], wq_d[4, :, :HK])
            for ki in range(1, 5):
                dma(xt_sb[:, ki, :], xt_d[ki * P:(ki + 1) * P, :])
            dma(wq_sb[4][:, HK:], wq_d[4, :, HK:])
            for ki in range(5, 9):
                dma(xt_sb[:, ki, :], xt_d[ki * P:(ki + 1) * P, :])
            dma(wq_sb[0][:, :HK], wq_d[0, :, :HK])
            for ki in range(9, 13):
                dma(xt_sb[:, ki, :], xt_d[ki * P:(ki + 1) * P, :])
            dma(wq_sb[0][:, HK:], wq_d[0, :, HK:])
            for ki in range(13, 18):
                dma(xt_sb[:, ki, :], xt_d[ki * P:(ki + 1) * P, :])
            dma(cos_sb[:, :HT], cos_d[:, :HT])
            dma(sin_sb[:, :HT], sin_d[:, :HT])
            for ki in range(18, KT):
                dma(xt_sb[:, ki, :], xt_d[ki * P:(ki + 1) * P, :])
            dma(cos_sb[:, HT:], cos_d[:, HT:])
            dma(sin_sb[:, HT:], sin_d[:, HT:])
            dma(mask_sb, mask_d)
            dma(esink_sb, esink_d)
            dma(bqkv_sb, bqkv_d)
            dma(wq_sb[1][:, :HK], wq_d[1, :, :HK])
            dma(wq_sb[1][:, HK:], wq_d[1, :, HK:])
            dma(wout_sb[0][:, :HO], wout_d[0, :, :HO])
            dma(wout_sb[0][:, HO:], wout_d[0, :, HO:])
            dma(wq_sb[2][:, :HK], wq_d[2, :, :HK])
            dma(wq_sb[2][:, HK:], wq_d[2, :, HK:])
            for kk in range(1, 4):
                dma(wout_sb[kk][:, :HO], wout_d[kk, :, :HO])
                dma(wout_sb[kk][:, HO:], wout_d[kk, :, HO:])
            dma(wq_sb[3][:, :HK], wq_d[3, :, :HK])
            dma(wq_sb[3][:, HK:], wq_d[3, :, HK:])

            # ---- sum of squares: vector bf16 squares + PE ones-reduction ----
            psum_ssq = [pY.tile([1, 512], dt.float32, tag="py", name=f"ssq{h}")
                        for h in range(2)]  # holds 2 of pY's 3 bufs until rsq
            for ki in range(KT):
                xsq = xsqp.tile([P, T], dt.bfloat16, tag="xsq", name="xsq")
                nc.vector.tensor_mul(xsq, xt_sb[:, ki, :], xt_sb[:, ki, :])
                for half in range(2):
                    nc.tensor.matmul(
                        psum_ssq[half],
                        ones,
                        xsq[:, half * 512:half * 512 + 512],
                        start=(ki == 0), stop=(ki == KT - 1),
                    )

            # rsq_b = exp(-0.5*ln(ssq/H + eps)) broadcast to 128 partitions
            lnm = res.tile([1, T], dt.float32, tag="lnm", name="lnm")
            for half in range(2):
                nc.scalar.activation(lnm[:, half * 512:half * 512 + 512],
                                     psum_ssq[half],
                                     mybir.ActivationFunctionType.Ln,
                                     bias=eps_t, scale=1.0 / HIDDEN)
            rinv = res.tile([1, T], dt.float32, tag="rinv", name="rinv")
            nc.scalar.activation(rinv, lnm, mybir.ActivationFunctionType.Exp,
                                 bias=zbias[0:1, :], scale=-0.5)
            rsq_b = res.tile([P, T], dt.float32, tag="rsq", name="rsq_b")
            nc.gpsimd.partition_broadcast(rsq_b, rinv)

            # ---- qkv projection (scaled+biased, bf16) ----
            def qkv_tile(n, half, dst):
                """dst <- (W x)*rsq + b for columns [half*512, ...+512)."""
                c0 = half * 512
                pq = pA.tile([P, 512], dt.float32, tag="pb", name="pb")
                for ki in range(KT):
                    nc.tensor.matmul(
                        pq,
                        wq_sb[n][:, ki * P:ki * P + P],
                        xt_sb[:, ki, c0:c0 + 512],
                        start=(ki == 0), stop=(ki == KT - 1),
                    )
                nc.vector.tensor_mul(dst, pq, rsq_b[:, c0:c0 + 512])
                nc.vector.tensor_scalar_add(dst, dst, bqkv_sb[:, n:n + 1])

            qra = res.tile([64, HPC, T], dt.bfloat16, tag="qra", name="qra")
            krope = res.tile([64, T], dt.bfloat16, tag="krope", name="krope")
            # all 8 token-major v tiles in one buffer; ones column prefilled
            vtok = res.tile([P, MT, AW], dt.bfloat16, tag="vtok", name="vtok")
            nc.vector.memset(vtok[:, :, HD:HD + 1], 1.0)

            qkvT4 = res.tile([P, T], dt.bfloat16, tag="qkvT4", name="qkvT4")

            def kv_phase(half):
                c0 = half * 512
                qkv_tile(4, half, qkvT4[:, c0:c0 + 512])
                for j in range(4):
                    b = half * 4 + j
                    pv = pS.tile([P, HD], dt.bfloat16, tag="sc", name="pv")
                    nc.tensor.transpose(pv, qkvT4[0:64, b * P:(b + 1) * P],
                                        identb[:64, :64])
                    nc.vector.tensor_copy(vtok[:, b, 0:HD], pv)
                # k rope (rows 64:128) at half width
                kswp = ropep.tile([P, 512], dt.bfloat16, tag="hswp", name="kswp")
                for a in (64, 96):
                    nc.scalar.copy(kswp[a:a + 32, :],
                                   qkvT4[a ^ 32:(a ^ 32) + 32, c0:c0 + 512])
                ktc = ropep.tile([P, 512], dt.bfloat16, tag="htc", name="ktc")
                nc.vector.tensor_mul(ktc[64:128, :], qkvT4[64:128, c0:c0 + 512],
                                     cos_sb[64:128, c0:c0 + 512])
                kts = ropep.tile([P, 512], dt.bfloat16, tag="hts", name="kts")
                nc.vector.tensor_mul(kts[64:128, :], kswp[64:128, :],
                                     sin_sb[64:128, c0:c0 + 512])
                nc.vector.tensor_add(krope[:, c0:c0 + 512], ktc[64:128, :],
                                     kts[64:128, :])

            def q_phase(half):
                c0 = half * 512
                for n in range(4):
                    qkvt = qkvp.tile([P, 512], dt.bfloat16, tag="qkvT",
                                     name=f"qkvt{n}")
                    qkv_tile(n, half, qkvt)
                    # rope at half width on the fly (swaps on the scalar queue)
                    swp = ropep.tile([P, 512], dt.bfloat16, tag="hswp", name="hswp")
                    for a in range(0, P, 32):
                        nc.scalar.copy(swp[a:a + 32, :],
                                       qkvt[a ^ 32:(a ^ 32) + 32, :])
                    tc_ = ropep.tile([P, 512], dt.bfloat16, tag="htc", name="htc")
                    nc.vector.tensor_mul(tc_, qkvt, cos_sb[:, c0:c0 + 512])
                    ts_ = ropep.tile([P, 512], dt.bfloat16, tag="hts", name="hts")
                    nc.vector.tensor_mul(ts_, swp, sin_sb[:, c0:c0 + 512])
                    for i in range(2):
                        b0 = 64 * i
                        nc.vector.tensor_add(qra[:, 2 * n + i, c0:c0 + 512],
                                             tc_[b0:b0 + 64, :],
                                             ts_[b0:b0 + 64, :])

            # ---- attention + out-projection for one token tile ----
            def attention(b):
                pt = ptp.tile([P, 2, HPC, P], dt.bfloat16, tag="pt", name="pt")
                kts = [(0, b - 1), (1, b)] if b > 0 else [(1, b)]
                for s, kt in kts:
                    for g in range(2):
                        ps = pS.tile([P, 4, P], dt.float32, tag="sc", name="sc")
                        nc.tensor.matmul(
                            ps,
                            krope[:, kt * P:(kt + 1) * P],
                            qra[:, 4 * g:4 * g + 4, b * P:(b + 1) * P],
                            start=True, stop=True)
                        nc.scalar.activation(pt[:, s, 4 * g:4 * g + 4, :], ps,
                                             mybir.ActivationFunctionType.Exp,
                                             bias=zbias, scale=SM_SCALE)
                    m0 = 0 if s == 0 else P
                    nc.vector.tensor_mul(pt[:, s], pt[:, s],
                                         bcast_mid(mask_sb[:, m0:m0 + P], HPC))

                # AV with ones column -> denominators in column 64
                rec8 = smallp.tile([P, HPC], dt.float32, tag="rec8", name="rec8")
                an = anp.tile([P, HPC, HD], dt.bfloat16, tag="an", name="an")
                for g in range(2):
                    pg = pS.tile([P, 4, AW], dt.float32, tag="sc", name="pg")
                    for j in range(4):
                        h = 4 * g + j
                        for idx, (s, kt) in enumerate(kts):
                            nc.tensor.matmul(pg[:, j, :], pt[:, s, h, :],
                                             vtok[:, kt, :],
                                             start=(idx == 0),
                                             stop=(idx == len(kts) - 1))
                    g0 = 4 * g
                    nc.vector.tensor_add(rec8[:, g0:g0 + 4],
                                         pg[:, :, HD:HD + 1],
                                         esink_sb[:, g0:g0 + 4])
                    nc.vector.reciprocal(rec8[:, g0:g0 + 4], rec8[:, g0:g0 + 4])
                    rec3 = bass.AP(tensor=rec8.tensor,
                                   offset=rec8[:, g0:g0 + 4].offset,
                                   ap=[rec8.ap[0], [1, 4], [0, HD]])
                    nc.vector.tensor_mul(an[:, g0:g0 + 4, :], pg[:, :, 0:HD],
                                         rec3)

                # transpose to out-proj lhsT layout [128 hd, 128 tok]
                att = attp.tile([P, 4, P], dt.bfloat16, tag="att", name="att")
                a2 = an.rearrange("p a b -> p (a b)")
                for kk in range(4):
                    pat = pS.tile([P, P], dt.bfloat16, tag="sc", name="pat")
                    nc.tensor.transpose(pat, a2[:, kk * P:(kk + 1) * P], identb)
                    if kk % 2 == 0:
                        nc.vector.tensor_copy(att[:, kk, :], pat)
                    else:
                        nc.scalar.copy(att[:, kk, :], pat)

                # out projection, bf16 out; y DMAs ride the scalar queue
                # (the last tile is drained in 480-chunks on both queues to
                # shorten the kernel tail)
                for hy in range(3):
                    ysb = ysbp.tile([P, 2, YC], dt.bfloat16, tag="ysb", name="ysb")
                    for j in range(2):
                        ch = 2 * hy + j
                        o0 = ch * YC
                        pyt = pY.tile([P, 512], dt.float32, tag="py", name="py")
                        for kk in range(4):
                            nc.tensor.matmul(pyt[:, 0:YC], att[:, kk, :],
                                             wout_sb[kk][:, o0:o0 + YC],
                                             start=(kk == 0), stop=(kk == 3))
                        if ch % 2 == 0:
                            nc.vector.tensor_copy(ysb[:, j, :], pyt[:, 0:YC])
                        else:
                            nc.scalar.copy(ysb[:, j, :], pyt[:, 0:YC])
                        if b == MT - 1:
                            eng = nc.sync if ch % 2 == 0 else nc.scalar
                            eng.dma_start(
                                out=y_d[b * P:(b + 1) * P, o0:o0 + YC],
                                in_=ysb[:, j, :])
                    if b < MT - 1:
                        nc.scalar.dma_start(
                            out=y_d[b * P:(b + 1) * P,
                                    hy * 2 * YC:(hy + 1) * 2 * YC],
                            in_=ysb)

            for half in range(2):
                kv_phase(half)
                q_phase(half)
                for j in range(4):
                    attention(half * 4 + j)

    nc.compile()
    return nc


# ----------------------------------------------------------------------------
# public entry
# ----------------------------------------------------------------------------

LAST_RESULTS = None


def kernel(x, norm_scale, qkv_w, qkv_b, out_w, out_b, sinks):
    global LAST_RESULTS
    x = np.asarray(x, dtype=np.float32)
    norm_scale = np.asarray(norm_scale, dtype=np.float32)
    qkv_w = np.asarray(qkv_w, dtype=np.float32)
    qkv_b = np.asarray(qkv_b, dtype=np.float32)
    out_w = np.asarray(out_w, dtype=np.float32)
    out_b = np.asarray(out_b, dtype=np.float32)
    sinks = np.asarray(sinks, dtype=np.float32)

    if "nc" not in _CACHE:
        _CACHE["nc"] = build_nc()
    nc = _CACHE["nc"]

    in_maps = [
        _prep_core_inputs(c, x, norm_scale, qkv_w, qkv_b, out_w, sinks)
        for c in range(NCORES)
    ]
    import os
    tmpdir = os.environ.get("BASS_TMPDIR") or None
    res = run_bass_kernel_spmd(nc, in_maps, core_ids=list(range(NCORES)),
                               tmpdir=tmpdir)
    LAST_RESULTS = res
    y = np.zeros((T, HIDDEN), dtype=np.float64)
    for c in range(NCORES):
        y += res.results[c]["y"].astype(np.float64)
    y += out_b.astype(np.float64)[None, :]
    return y.astype(np.float32)
